# revision 2
# baseline (speedup 1.0000x reference)
"""Trainium2 Bass kernel for the differentiable gaussian-splat renderer.

Full-input contract: kernel(**inputs) takes the unsharded inputs and returns
the full [2*16, 3, 32, 32] output.

Math (per pose):
    cam = positions @ R.T + t ;  pj = (fx*cam_x/cam_z + cx, fy*cam_y/cam_z + cy)
    w[n, p] = op_n * exp(-0.5*((px-ax_n)^2 + (py-ay_n)^2)/s_n^2)
    img = (w.T @ colors) / (w.T @ 1 + 1e-8)

Two key structural ideas:

1. Runtime pruning. The weights are separable gaussians whose peak in-image
   value spans hundreds of e-folds across n. Any gaussian whose peak
   log-weight is more than MARGIN=75 below the pose's max contributes a
   relative error < e^-58 to every output pixel (even summed over all N and
   all HW pixels), far below the tolerance. The host computes each gaussian's
   in-image peak log-weight in O(N) and keeps only the significant ones,
   padded up to K = J*128. For typical scenes J is 1-4, cutting the device
   work by 8-30x.

2. Colors folded into the exponent. Instead of materializing
   X = color (*) wx with a DVE pass over an HBM-expanded color array,
   ln(color_c) is added to the per-(chunk, c-block) constant coefficient so
   the arg matmul directly yields arg_x[n, (j, c, px)] and one exp produces
   X = exp(quad(px) + ln c) = color_c * wx. The den block (c=3) has no color
   rows. This removes the 1 MiB color expansion DMA and the DVE multiply.

Sharding: 8 cores = 2 poses x 4 px-column blocks (32 px each). No
collectives. Each core produces PSUM po[py, 32c+px] = (num | den), copied to
SBUF and DMA'd out; the host does the final num/(den+1e-8) (O(HW) work).

Per-core device program (J chunks of 128 kept gaussians):
    arg_x matmul:  lhsT = coef_x pack [18*cg, 128], rhs = block-diag basis
                   [18*cg, 128*cg]  -> pa_x[n, (j, c, px)]   (PSUM)
    arg_y matmul:  [16*cg, 128] x [16*cg, 128*cg] -> pa_y[n, (j, py)]
    exp (scalar):  X = exp(pa_x) bf16 ; wy = exp(pa_y) bf16
    acc (PE):      po[py, (c,px)] += wy_chunk.T @ X_chunk     (J matmuls)
    copy (DVE):    img = po  (PSUM -> SBUF)
    out DMA:       img halves via the Sync and GpSimd HWDGE queues (both
                   rings pre-warmed by the two input DMAs).
"""

import numpy as np

H = 128
W = 128
FX = 120.0
FY = 120.0
CX = 64.0
CY = 64.0
N = 4096
NPOSE = 2
PXB = 32             # px columns per core
NBLK = 4             # px blocks
F32 = np.float32

MARGIN = 75.0        # keep peak_logw >= pose_max - MARGIN   (error ~ e^-58)
RX = 18              # coef rows per chunk, x/color pack (12 quad + 6 lncolor)
RY = 16              # coef rows per chunk, y pack (12 quad + 4 pad)
GX = 7               # max chunks per x matmul group (7*18 = 126 <= 128)
GY = 8               # max chunks per y matmul group (8*16 = 128)

_CACHE = {}


def _quat2mat(q):
    q = np.asarray(q, dtype=np.float64)
    q = q / np.linalg.norm(q)
    w, x, y, z = q
    return np.array([
        [1 - 2 * (y * y + z * z), 2 * (x * y - z * w), 2 * (x * z + y * w)],
        [2 * (x * y + z * w), 1 - 2 * (x * x + z * z), 2 * (y * z - x * w)],
        [2 * (x * z - y * w), 2 * (y * z + x * w), 1 - 2 * (x * x + y * y)],
    ])


def _groups(J, gmax):
    """Split J chunks into groups of at most gmax: [(start, count), ...]."""
    out = []
    s = 0
    while s < J:
        c = min(gmax, J - s)
        out.append((s, c))
        s += c
    return out


def _layout(J):
    """Column layout of the single per-core 'bas' DRAM tensor.

    Returns (PB, CB, xsegs, ysegs) where xsegs[g] = (bas_off, coef_off,
    chunk_start, cg) and similarly ysegs."""
    off = 0
    xsegs = []
    for s, cg in _groups(J, GX):
        xsegs.append((off, off + 128 * cg, s, cg))
        off += 128 * cg + 128
    ysegs = []
    for s, cg in _groups(J, GY):
        ysegs.append((off, off + 128 * cg, s, cg))
        off += 128 * cg + 128
    PB = max(RX * min(J, GX), RY * min(J, GY))
    return PB, off, xsegs, ysegs


def _build_program(J):
    """Build the SPMD Bass/Tile program for J chunks (same on every core)."""
    import concourse.bacc as bacc
    import concourse.tile as tile
    import concourse.mybir as mybir
    from contextlib import ExitStack

    dt = mybir.dt.float32
    bf = mybir.dt.bfloat16
    PB, CB, xsegs, ysegs = _layout(J)
    nc = bacc.Bacc()

    # split the input across the Sync and GpSimd HWDGE queues so descriptor
    # generation runs in parallel and both rings are warm for the output
    xcols = ysegs[0][0]
    basx_d = nc.dram_tensor("basx", [PB, xcols], bf, kind="ExternalInput").ap()
    basy_d = nc.dram_tensor("basy", [PB, CB - xcols], bf,
                            kind="ExternalInput").ap()
    out_d = nc.dram_tensor("out", [128, 128], dt, kind="ExternalOutput").ap()

    add = mybir.AluOpType.add
    EXP = mybir.ActivationFunctionType.Exp

    with tile.TileContext(nc) as tc, ExitStack() as ctx:
        const = ctx.enter_context(tc.tile_pool(name="const", bufs=1))
        psum_arg = ctx.enter_context(tc.tile_pool(name="pa", bufs=3,
                                                  space="PSUM"))
        psum_out = ctx.enter_context(tc.tile_pool(name="po", bufs=1,
                                                  space="PSUM"))

        po = psum_out.tile([128, 128], dt, tag="po")

        bas = const.tile([128, CB], bf, tag="bas")
        nc.sync.dma_start(out=bas[0:PB, 0:xcols], in_=basx_d)
        nc.gpsimd.dma_start(out=bas[0:PB, xcols:CB], in_=basy_d)

        X = const.tile([128, 128 * J], bf, tag="X")
        wy = const.tile([128, 128 * J], bf, tag="wy")

        # interleave x/y arg matmuls and exps group-wise
        nseg = max(len(xsegs), len(ysegs))
        for i in range(nseg):
            if i < len(xsegs):
                boff, coff, s, cg = xsegs[i]
                pa = psum_arg.tile([128, 128 * cg], dt, tag="pa")
                nc.tensor.matmul(pa[:],
                                 lhsT=bas[0:RX * cg, coff:coff + 128],
                                 rhs=bas[0:RX * cg, boff:boff + 128 * cg],
                                 start=True, stop=True)
                nc.scalar.activation(out=X[:, 128 * s:128 * (s + cg)],
                                     in_=pa[:], func=EXP)
            if i < len(ysegs):
                boff, coff, s, cg = ysegs[i]
                pa = psum_arg.tile([128, 128 * cg], dt, tag="pa")
                nc.tensor.matmul(pa[:],
                                 lhsT=bas[0:RY * cg, coff:coff + 128],
                                 rhs=bas[0:RY * cg, boff:boff + 128 * cg],
                                 start=True, stop=True)
                nc.scalar.activation(out=wy[:, 128 * s:128 * (s + cg)],
                                     in_=pa[:], func=EXP)

        for j in range(J):
            nc.tensor.matmul(po[:],
                             lhsT=wy[:, 128 * j:128 * j + 128],
                             rhs=X[:, 128 * j:128 * j + 128],
                             start=(j == 0), stop=(j == J - 1))

        img = const.tile([128, 128], dt, tag="img")
        nc.vector.tensor_scalar(out=img[:], in0=po[:], scalar1=0.0,
                                scalar2=None, op0=add)
        nc.sync.dma_start(out=out_d[:, 0:64], in_=img[:, 0:64])
        nc.gpsimd.dma_start(out=out_d[:, 64:128], in_=img[:, 64:128])

    nc.compile()
    return nc


def _split3(v, bf):
    """Exact-ish 3-way bf16 split of a float64 array v."""
    v = v.astype(F32)
    p1 = v.astype(bf)
    r1 = (v - p1.astype(F32)).astype(F32)
    p2 = r1.astype(bf)
    r2 = (r1 - p2.astype(F32)).astype(F32)
    p3 = r2.astype(bf)
    return p1, p2, p3


def _split2(v, bf):
    v = v.astype(F32)
    p1 = v.astype(bf)
    p2 = (v - p1.astype(F32)).astype(F32).astype(bf)
    return p1, p2


def _basis12(q):
    """[12, len(q)] f64->bf16 rows: p2h,p2l,p2h,p2l,p2h,p2l,q,q,q,1,1,1."""
    import ml_dtypes
    bf = ml_dtypes.bfloat16
    q = q.astype(F32)
    p2 = (q * q).astype(F32)
    p2h = p2.astype(bf)
    p2l = (p2 - p2h.astype(F32)).astype(F32).astype(bf)
    qb = q.astype(bf)
    one = np.ones_like(q, dtype=bf)
    return np.stack([p2h, p2l, p2h, p2l, p2h, p2l, qb, qb, qb, one, one, one])


def _host_prep(positions, colors, opacities, scales, qvec, tvec):
    """O(N) host prep: prune, build per-core coef/basis packs."""
    import ml_dtypes
    bf = ml_dtypes.bfloat16

    positions = np.asarray(positions, dtype=np.float64)
    colors = np.asarray(colors, dtype=np.float64)
    opacities = np.asarray(opacities, dtype=np.float64)
    scales = np.asarray(scales, dtype=np.float64)
    qvec = np.asarray(qvec, dtype=F32)
    tvec = np.asarray(tvec, dtype=F32)

    var = scales[:, 0] ** 2
    lnop = np.log(np.maximum(opacities[:, 0], 1e-300))
    lncol = np.log(np.maximum(colors, 1e-12))          # [N, 3]

    # project + prune per pose
    poses = []
    for p in range(NPOSE):
        R = _quat2mat(qvec[p])
        t64 = tvec[p].astype(np.float64)
        u = positions @ (FX * R[0]) + FX * t64[0]
        v = positions @ (FY * R[1]) + FY * t64[1]
        zc = positions @ R[2] + t64[2]
        ax = u / zc + CX
        ay = v / zc + CY
        dx = np.maximum.reduce([0.0 - ax, ax - (W - 1), np.zeros(N)])
        dy = np.maximum.reduce([0.0 - ay, ay - (H - 1), np.zeros(N)])
        peak = lnop - 0.5 * (dx * dx + dy * dy) / var
        keep = np.where(peak >= peak.max() - MARGIN)[0]
        keep = keep[np.argsort(-peak[keep])]
        poses.append((ax, ay, keep))

    K = max(len(poses[0][2]), len(poses[1][2]), 1)
    K = -(-K // 128) * 128
    J = K // 128
    PB, CB, xsegs, ysegs = _layout(J)
    xcols = ysegs[0][0]

    py = np.arange(128) - CY
    by_rows = _basis12(py)                              # [12, 128]

    in_maps = []
    for p in range(NPOSE):
        ax, ay, keep = poses[p]
        nk = len(keep)
        g_k = -0.5 / var[keep]
        ayc = ay[keep] - CY

        # ---- y pack rows [RY, K]: quad coefs for ayc, padded slots -> -6e4
        def coef_rows(A, B, C, LC, nrows):
            """[nrows, K] bf16: 12 quad rows (+6 lncolor rows if LC)."""
            a1, a2, a3 = _split3(A, bf)
            b1, b2, b3 = _split3(B, bf)
            c1, c2, c3 = _split3(C, bf)
            rows = [a1, a1, a2, a2, a3, a3, b1, b2, b3, c1, c2, c3]
            if LC is not None:
                for c in range(3):
                    l1, l2 = _split2(LC[:, c], bf)
                    rows += [l1, l2]
            rows = np.stack(rows)                       # [12 or 18, nk]
            out = np.zeros((nrows, K), bf)
            out[:rows.shape[0], :nk] = rows
            out[9, nk:] = bf(-60000.0)                  # c1 row: exp -> 0
            return out

        cy_rows = coef_rows(g_k, -2.0 * g_k * ayc, g_k * ayc * ayc,
                            None, RY)                   # [RY, K]

        for b in range(NBLK):
            cb = 32.0 * b + 16.0
            axc = ax[keep] - cb
            cx_rows = coef_rows(g_k, -2.0 * g_k * axc,
                                g_k * axc * axc + lnop[keep],
                                lncol[keep], RX)        # [RX, K]
            px = np.arange(PXB) - 16.0                  # block-local px
            bx_rows = _basis12(px)                      # [12, 32]

            bas = np.zeros((PB, CB), bf)
            for boff, coff, s, cg in xsegs:
                for i in range(cg):
                    j = s + i
                    r0 = RX * i
                    c0 = boff + 128 * i
                    # basis block: quad rows tiled over the 4 c-blocks
                    for c in range(4):
                        bas[r0:r0 + 12, c0 + 32 * c:c0 + 32 * c + 32] = bx_rows
                        if c < 3:
                            bas[r0 + 12 + 2 * c:r0 + 14 + 2 * c,
                                c0 + 32 * c:c0 + 32 * c + 32] = bf(1.0)
                    bas[r0:r0 + RX, coff:coff + 128] = \
                        cx_rows[:, 128 * j:128 * j + 128]
            for boff, coff, s, cg in ysegs:
                for i in range(cg):
                    j = s + i
                    r0 = RY * i
                    c0 = boff + 128 * i
                    bas[r0:r0 + 12, c0:c0 + 128] = by_rows
                    bas[r0:r0 + RY, coff:coff + 128] = \
                        cy_rows[:, 128 * j:128 * j + 128]
            in_maps.append({
                "basx": np.ascontiguousarray(bas[:, :xcols]),
                "basy": np.ascontiguousarray(bas[:, xcols:]),
            })
    return in_maps, J


def _assemble(slabs):
    """slabs: 8 x [128, 128] (num|den) -> [NPOSE*16, 3, 32, 32] output."""
    out = []
    for p in range(NPOSE):
        img = np.zeros((H, W, 3), F32)
        for b in range(NBLK):
            slab = slabs[p * NBLK + b].astype(np.float64)
            den = slab[:, 96:128] + 1e-8                # [128 py, 32 px]
            for c in range(3):
                img[:, PXB * b:PXB * b + PXB, c] = \
                    (slab[:, 32 * c:32 * c + 32] / den).astype(F32)
        tiles = img.reshape(H * W, 3).reshape(16, 1024, 3)
        tiles = tiles.transpose(0, 2, 1).reshape(16, 3, 32, 32)
        out.append(tiles)
    return np.concatenate(out, axis=0).astype(F32)


def _with_backend_flags():
    """Append walrus backend options for this compile; returns restore fn."""
    import libneuronxla.libncc as ncc
    orig = list(ncc.NEURON_CC_FLAGS)
    flags = list(orig)
    for i, f in enumerate(flags):
        if f.startswith("--internal-backend-options=") and \
                "--max-sem-num" not in f:
            flags[i] = f + " --max-sem-num=16"
    ncc.NEURON_CC_FLAGS = flags

    def restore():
        ncc.NEURON_CC_FLAGS = orig
    return restore


def kernel(positions, colors, opacities, scales, qvec, tvec, _trace=False):
    from concourse.bass_utils import run_bass_kernel_spmd

    in_maps, J = _host_prep(positions, colors, opacities, scales, qvec, tvec)
    if ("nc", J) not in _CACHE:
        _CACHE[("nc", J)] = _build_program(J)
    nc = _CACHE[("nc", J)]

    restore = _with_backend_flags()
    try:
        res = run_bass_kernel_spmd(nc, in_maps, core_ids=list(range(8)),
                                   trace=_trace)
    finally:
        restore()
    slabs = [np.asarray(res.results[c]["out"]) for c in range(8)]
    out = _assemble(slabs)
    if _trace:
        _CACHE["last_result"] = res
    return out
